# revision 13
# baseline (speedup 1.0000x reference)
"""Trainium2 Bass kernel for nn_AdditiveCoupling (NICE additive coupling layer).

reference math:
    first  = x[:, 0::2]            # (B, 392)
    second = x[:, 1::2]            # (B, 392)
    shift  = MLP(first)            # 392 -> 1000 -> (1000)x4 -> 392, ReLU between
    y[:, 0::2] = first
    y[:, 1::2] = second + shift    # i.e. y = x with shift added to odd columns
    returns (y, log_det_J)         # log_det_J passes through unchanged

Strategy: data-parallel over 8 NeuronCores. Each core takes a 512-row batch
shard and the full (replicated) weights; no inter-core communication.

Compute (per core): activations kept feature-major ([feat, batch]) so every
layer is matmul(psum[M,N] = W_chunk[K,M].T @ A[K,N]) with the weight chunk as
the stationary operand; contraction chunks of 125 (98 for the 392-wide first
layer). Layers run k-outer/m-inner with all 8 PSUM banks as concurrent
accumulation groups so each weight K-tile's SBUF slot frees right after its
k-pass. PSUM eviction fuses bias+ReLU on ScalarE. The last layer swaps
operands (lhsT = activation chunk) so its output lands batch-major [128, 392]
in PSUM; its bias comes via a ones-row matmul, and one strided VectorE add
folds the shift into x's odd columns in-place before the output DMA. Matmuls
run in bf16 (f32r measured only ~1.3GHz row streaming; bf16 streams at the
full PE clock): rel l2 err ~5e-4, well under the 2e-2 gate.

DMA plan (the critical resource — 22.4MB must move at ~300+GB/s): three
independent queues are kept busy in parallel:
  - gpsimd SWDGE: cast-DMAs (fp32 DRAM -> bf16 SBUF inline) for x-chunks 0/3,
    W_in, W_hid k-tiles 0-3, W_out (SWDGE sustains ~180-240GB/s alone),
  - sync HWDGE: x1, biases, W_hid k-tiles 4/5 as fp32 into staging tiles,
    output chunks 0/2,
  - scalar HWDGE: x2, W_hid k-tiles 6/7 staging, output chunks 1/3.
Staged fp32 tiles are cast to bf16 by otherwise-idle VectorE copies. Weight
slots are triple-buffered so transfers never wait on slot release and the PE
never starves (a starved PE re-enters the 1.2GHz HAM-throttled clock state).
"""

import sys

sys.path.insert(0, "/opt/trn_rl_repo")

import numpy as np

import concourse.bass as bass  # noqa: F401  (engine types via nc)
import concourse.tile as tile
from concourse import bacc, mybir
from concourse.bass_utils import run_bass_kernel_spmd
from concourse.masks import make_identity

N_CORES = 8
B, D, MID = 4096, 784, 1000
HALF = D // 2  # 392
BS = B // N_CORES  # 512 rows per core
NB = BS // 128  # 4 batch tiles per core

KH = 98  # feature chunk for the 392-wide dims (4 chunks)
KM = 125  # feature chunk for the 1000-wide dims (8 chunks)
NH = HALF // KH  # 4
NM = MID // KM  # 8
N_CAST_DMA = 4  # whid k-tiles 0..3 via gpsimd cast-DMA; 4..7 staged via HWDGE

F32 = mybir.dt.float32
BF16 = mybir.dt.bfloat16
RELU = mybir.ActivationFunctionType.Relu

_CACHED_NC = None


def build_nc():
    nc = bacc.Bacc("TRN2", target_bir_lowering=False, debug=False)

    x = nc.declare_dram_parameter("x", [BS, D], F32, isOutput=False)
    w_in = nc.declare_dram_parameter("W_in", [HALF, MID], F32, isOutput=False)
    b_in = nc.declare_dram_parameter("b_in", [MID], F32, isOutput=False)
    w_hid = nc.declare_dram_parameter("W_hid", [4, MID, MID], F32, isOutput=False)
    b_hid = nc.declare_dram_parameter("b_hid", [4, MID], F32, isOutput=False)
    w_out = nc.declare_dram_parameter("W_out", [MID, HALF], F32, isOutput=False)
    b_out = nc.declare_dram_parameter("b_out", [HALF], F32, isOutput=False)
    out = nc.declare_dram_parameter("out", [BS, D], F32, isOutput=True)

    with tile.TileContext(nc) as tc:
        with (
            tc.tile_pool(name="const", bufs=1) as constp,
            tc.tile_pool(name="xp", bufs=1) as xp,
            tc.tile_pool(name="winp", bufs=1) as winp,
            tc.tile_pool(name="whidp", bufs=3) as whidp,
            tc.tile_pool(name="wstag", bufs=2) as wstag,
            tc.tile_pool(name="woutp", bufs=1) as woutp,
            tc.tile_pool(name="actp", bufs=2) as actp,
            tc.tile_pool(name="biasp", bufs=1) as biasp,
            tc.tile_pool(name="psp", bufs=8, space="PSUM") as psp,
        ):
            ident = constp.tile([128, 128], F32, name="ident", tag="ident")
            make_identity(nc, ident)
            ones = constp.tile([1, 128], BF16, name="ones", tag="ones")
            nc.vector.memset(ones, 1.0)

            # ---- input: 4 separate tiles; chunks spread over all 3 queues ----
            xv = x.rearrange("(b p) d -> b p d", p=128)
            X = []
            x_eng = [nc.gpsimd, nc.sync, nc.scalar, nc.gpsimd]
            for b in range(NB):
                xt = xp.tile([128, D], F32, name=f"x{b}", tag=f"x{b}")
                x_eng[b].dma_start(out=xt, in_=xv[b])
                X.append(xt)

            # ---- W_in: 4 cast-DMAs on the SWDGE queue ----
            WIN = []
            for k in range(NH):
                wt = winp.tile([KH, MID], BF16, name=f"win{k}", tag=f"win{k}")
                nc.gpsimd.dma_start(out=wt, in_=w_in[k * KH : (k + 1) * KH, :])
                WIN.append(wt)
            bin_t = biasp.tile([KM, NM], F32, name="bin", tag="bin")
            nc.sync.dma_start(out=bin_t, in_=b_in.rearrange("(m p) -> p m", p=KM))

            def load_whid(layer):
                """k 0..3: gpsimd cast-DMA. k 4..7: fp32 staging on sync/scalar
                HWDGE + VectorE cast (DVE is idle mid-kernel)."""
                tiles = []
                for k in range(NM):
                    wt = whidp.tile(
                        [KM, MID], BF16, name=f"wh{layer}_{k}", tag=f"wh{k}"
                    )
                    src = w_hid[layer, k * KM : (k + 1) * KM, :]
                    if k < N_CAST_DMA:
                        nc.gpsimd.dma_start(out=wt, in_=src)
                    else:
                        st = wstag.tile(
                            [KM, MID], F32, name=f"ws{layer}_{k}", tag=f"ws{k}"
                        )
                        eng = nc.sync if k < 6 else nc.scalar
                        eng.dma_start(out=st, in_=src)
                        nc.vector.tensor_copy(wt, st)
                    tiles.append(wt)
                return tiles

            WH0 = load_whid(0)
            bhid_t = []
            for i in range(4):
                bt = biasp.tile([KM, NM], F32, name=f"bh{i}", tag=f"bh{i}")
                nc.sync.dma_start(
                    out=bt, in_=b_hid[i].rearrange("(m p) -> p m", p=KM)
                )
                bhid_t.append(bt)

            # ---- split even columns + transpose to feature-major ----
            # A0[f] = first.T chunk f: [98 feats, 512 batch]
            A0 = [
                actp.tile([KH, BS], BF16, name=f"A0_{f}", tag=f"a{f}")
                for f in range(NH)
            ]
            for b in range(NB):
                xb_pairs = X[b].rearrange("p (d two) -> p d two", two=2)
                for f in range(NH):
                    pt = psp.tile([KH, 128], F32, name=f"pt{b}_{f}", tag="bank")
                    nc.tensor.transpose(
                        pt, xb_pairs[:, f * KH : (f + 1) * KH, 0], ident
                    )
                    nc.vector.tensor_copy(A0[f][:, b * 128 : (b + 1) * 128], pt)

            def layer_fwd(A_prev, W, bias_col, nk, name):
                """k-outer/m-inner layer: all NM psum banks accumulate at once;
                weight K-tile k is fully consumed after its k-pass."""
                ps = [
                    psp.tile([KM, BS], F32, name=f"ps{name}_{m}", tag="bank")
                    for m in range(NM)
                ]
                for k in range(nk):
                    for m in range(NM):
                        nc.tensor.matmul(
                            ps[m],
                            W[k][:, m * KM : (m + 1) * KM],
                            A_prev[k],
                            start=(k == 0),
                            stop=(k == nk - 1),
                        )
                A_next = []
                for m in range(NM):
                    at = actp.tile(
                        [KM, BS], BF16, name=f"A{name}_{m}", tag=f"a{m}"
                    )
                    nc.scalar.activation(at, ps[m], RELU, bias=bias_col[:, m : m + 1])
                    A_next.append(at)
                return A_next

            # ---- layer 1: 392 -> 1000, ReLU ----
            A_prev = layer_fwd(A0, WIN, bin_t, NH, "1")

            # ---- hidden layers: 1000 -> 1000, ReLU ----
            WH = WH0
            for layer in range(4):
                if layer < 3:
                    WH_next = load_whid(layer + 1)
                else:
                    WH_next = None
                    # last hidden layer in flight: W_out / b_out loads go out now
                    WOUT = []
                    for k in range(NM):
                        wt = woutp.tile(
                            [KM, HALF], BF16, name=f"wo{k}", tag=f"wo{k}"
                        )
                        nc.gpsimd.dma_start(
                            out=wt, in_=w_out[k * KM : (k + 1) * KM, :]
                        )
                        WOUT.append(wt)
                    bout_t = biasp.tile([1, HALF], BF16, name="bout", tag="bout")
                    nc.gpsimd.dma_start(out=bout_t, in_=b_out[:].unsqueeze(0))
                A_prev = layer_fwd(A_prev, WH, bhid_t[layer], NM, f"h{layer}")
                WH = WH_next

            # ---- last layer: 1000 -> 392, batch-major output; add to odd cols ----
            ov = out.rearrange("(b p) d -> b p d", p=128)
            for b in range(NB):
                pso = psp.tile([128, HALF], F32, name=f"pso{b}", tag="bank")
                for k in range(NM):
                    nc.tensor.matmul(
                        pso,
                        A_prev[k][:, b * 128 : (b + 1) * 128],
                        WOUT[k],
                        start=(k == 0),
                        stop=False,
                    )
                # bias via ones-row: pso += ones.T @ b_out_row
                nc.tensor.matmul(pso, ones, bout_t, start=False, stop=True)
                xb_odd = X[b].rearrange("p (d two) -> p d two", two=2)[:, :, 1]
                nc.vector.tensor_add(xb_odd, xb_odd, pso)
                eng = nc.sync if b % 2 == 0 else nc.scalar
                eng.dma_start(out=ov[b], in_=X[b])

    nc.finalize()  # Bacc register allocation + freeze (bass2jax won't do it)
    return nc


def get_nc():
    global _CACHED_NC
    if _CACHED_NC is None:
        _CACHED_NC = build_nc()
    return _CACHED_NC


def run(inputs, trace=False, tmpdir=None):
    nc = get_nc()
    f32c = lambda a: np.ascontiguousarray(np.asarray(a), dtype=np.float32)
    x = f32c(inputs["x"])
    shared = {
        k: f32c(inputs[k])
        for k in ("W_in", "b_in", "W_hid", "b_hid", "W_out", "b_out")
    }
    in_maps = [
        {"x": x[i * BS : (i + 1) * BS], **shared} for i in range(N_CORES)
    ]
    res = run_bass_kernel_spmd(
        nc, in_maps, core_ids=list(range(N_CORES)), trace=trace, tmpdir=tmpdir
    )
    y = np.concatenate([res.results[i]["out"] for i in range(N_CORES)], axis=0)
    return y, res


def kernel(**inputs):
    y, _ = run(inputs)
    log_det_J = np.asarray(inputs["log_det_J"], dtype=np.float32)
    return y, log_det_J


# revision 14
# speedup vs baseline: 1.5971x; 1.5971x over previous
"""Trainium2 Bass kernel for nn_AdditiveCoupling (NICE additive coupling layer).

reference math:
    first  = x[:, 0::2]            # (B, 392)
    second = x[:, 1::2]            # (B, 392)
    shift  = MLP(first)            # 392 -> 1000 -> (1000)x4 -> 392, ReLU between
    y[:, 0::2] = first
    y[:, 1::2] = second + shift    # i.e. y = x with shift added to odd columns
    returns (y, log_det_J)         # log_det_J passes through unchanged

Strategy: data-parallel over 8 NeuronCores. Each core takes a 512-row batch
shard and the full (replicated) weights; no inter-core communication.

Compute (per core): activations kept feature-major ([feat, batch]) so every
layer is matmul(psum[M,N] = W_chunk[K,M].T @ A[K,N]) with the weight chunk as
the stationary operand; contraction chunks of 125 (98 for the 392-wide first
layer). Layers run k-outer/m-inner with all 8 PSUM banks as concurrent
accumulation groups so each weight K-tile's SBUF slot frees right after its
k-pass. PSUM eviction fuses bias+ReLU on ScalarE. The last layer swaps
operands (lhsT = activation chunk) so its output lands batch-major [128, 392]
in PSUM; its bias comes via a ones-row matmul, and one strided VectorE add
folds the shift into x's odd columns in-place before the output DMA. Matmuls
run in bf16 (f32r measured only ~1.3GHz row streaming; bf16 streams at the
full PE clock): rel l2 err ~5e-4, well under the 2e-2 gate.

DMA plan (the critical resource — 22.4MB must move at ~300+GB/s): three
independent queues are kept busy in parallel:
  - gpsimd SWDGE: cast-DMAs (fp32 DRAM -> bf16 SBUF inline) for x-chunks 0/3,
    W_in, W_hid k-tiles 0-3, W_out (SWDGE sustains ~180-240GB/s alone),
  - sync HWDGE: x1, biases, W_hid k-tiles 4/5 as fp32 into staging tiles,
    output chunks 0/2,
  - scalar HWDGE: x2, W_hid k-tiles 6/7 staging, output chunks 1/3.
Staged fp32 tiles are cast to bf16 by otherwise-idle VectorE copies. Weight
slots are triple-buffered so transfers never wait on slot release and the PE
never starves (a starved PE re-enters the 1.2GHz HAM-throttled clock state).
"""

import sys

sys.path.insert(0, "/opt/trn_rl_repo")

import numpy as np

import concourse.bass as bass  # noqa: F401  (engine types via nc)
import concourse.tile as tile
from concourse import bacc, mybir
from concourse.bass_utils import run_bass_kernel_spmd
from concourse.masks import make_identity

N_CORES = 8
B, D, MID = 4096, 784, 1000
HALF = D // 2  # 392
BS = B // N_CORES  # 512 rows per core
NB = BS // 128  # 4 batch tiles per core

KH = 98  # feature chunk for the 392-wide dims (4 chunks)
KM = 125  # feature chunk for the 1000-wide dims (8 chunks)
NH = HALF // KH  # 4
NM = MID // KM  # 8
N_CAST_DMA = 4  # whid k-tiles 0..3 via gpsimd cast-DMA; 4..7 staged via HWDGE

F32 = mybir.dt.float32
BF16 = mybir.dt.bfloat16
RELU = mybir.ActivationFunctionType.Relu

_CACHED_NC = None

_SWDGE_QUEUES = ["qPoolDynamic", "qPoolDynamic1", "qPoolDynamic2", "qPoolDynamic3"]
_swdge_rr = [0]


def _gp_dma(nc, out, in_):
    inst = nc.gpsimd.dma_start(out=out, in_=in_)
    inst.ins.queue = _SWDGE_QUEUES[_swdge_rr[0] % len(_SWDGE_QUEUES)]
    _swdge_rr[0] += 1
    return inst


def build_nc():
    _swdge_rr[0] = 0
    nc = bacc.Bacc("TRN2", target_bir_lowering=False, debug=False, num_swdge_queues=4)

    x = nc.declare_dram_parameter("x", [BS, D], F32, isOutput=False)
    w_in = nc.declare_dram_parameter("W_in", [HALF, MID], F32, isOutput=False)
    b_in = nc.declare_dram_parameter("b_in", [MID], F32, isOutput=False)
    w_hid = nc.declare_dram_parameter("W_hid", [4, MID, MID], F32, isOutput=False)
    b_hid = nc.declare_dram_parameter("b_hid", [4, MID], F32, isOutput=False)
    w_out = nc.declare_dram_parameter("W_out", [MID, HALF], F32, isOutput=False)
    b_out = nc.declare_dram_parameter("b_out", [HALF], F32, isOutput=False)
    out = nc.declare_dram_parameter("out", [BS, D], F32, isOutput=True)

    with tile.TileContext(nc) as tc:
        with (
            tc.tile_pool(name="const", bufs=1) as constp,
            tc.tile_pool(name="xp", bufs=1) as xp,
            tc.tile_pool(name="winp", bufs=1) as winp,
            tc.tile_pool(name="whidp", bufs=3) as whidp,
            tc.tile_pool(name="woutp", bufs=1) as woutp,
            tc.tile_pool(name="actp", bufs=2) as actp,
            tc.tile_pool(name="biasp", bufs=1) as biasp,
            tc.tile_pool(name="psp", bufs=8, space="PSUM") as psp,
        ):
            ident = constp.tile([128, 128], F32, name="ident", tag="ident")
            make_identity(nc, ident)
            ones = constp.tile([1, 128], BF16, name="ones", tag="ones")
            nc.vector.memset(ones, 1.0)

            # ---- input: 4 separate tiles; chunks spread over all 3 queues ----
            xv = x.rearrange("(b p) d -> b p d", p=128)
            X = []
            x_eng = [nc.sync, nc.scalar, nc.sync, nc.scalar]
            for b in range(NB):
                xt = xp.tile([128, D], F32, name=f"x{b}", tag=f"x{b}")
                x_eng[b].dma_start(out=xt, in_=xv[b])
                X.append(xt)

            # ---- W_in: 4 cast-DMAs on the SWDGE queue ----
            WIN = []
            for k in range(NH):
                wt = winp.tile([KH, MID], BF16, name=f"win{k}", tag=f"win{k}")
                _gp_dma(nc, wt, w_in[k * KH : (k + 1) * KH, :])
                WIN.append(wt)
            bin_t = biasp.tile([KM, NM], F32, name="bin", tag="bin")
            nc.sync.dma_start(out=bin_t, in_=b_in.rearrange("(m p) -> p m", p=KM))

            def load_whid(layer):
                """k 0..3: gpsimd cast-DMA. k 4..7: fp32 staging on sync/scalar
                HWDGE + VectorE cast (DVE is idle mid-kernel)."""
                tiles = []
                for k in range(NM):
                    wt = whidp.tile(
                        [KM, MID], BF16, name=f"wh{layer}_{k}", tag=f"wh{k}"
                    )
                    _gp_dma(nc, wt, w_hid[layer, k * KM : (k + 1) * KM, :])
                    tiles.append(wt)
                return tiles

            WH0 = load_whid(0)
            bhid_t = []
            for i in range(4):
                bt = biasp.tile([KM, NM], F32, name=f"bh{i}", tag=f"bh{i}")
                nc.sync.dma_start(
                    out=bt, in_=b_hid[i].rearrange("(m p) -> p m", p=KM)
                )
                bhid_t.append(bt)

            # ---- split even columns + transpose to feature-major ----
            # A0[f] = first.T chunk f: [98 feats, 512 batch]
            A0 = [
                actp.tile([KH, BS], BF16, name=f"A0_{f}", tag=f"a{f}")
                for f in range(NH)
            ]
            for b in range(NB):
                xb_pairs = X[b].rearrange("p (d two) -> p d two", two=2)
                for f in range(NH):
                    pt = psp.tile([KH, 128], F32, name=f"pt{b}_{f}", tag="bank")
                    nc.tensor.transpose(
                        pt, xb_pairs[:, f * KH : (f + 1) * KH, 0], ident
                    )
                    nc.vector.tensor_copy(A0[f][:, b * 128 : (b + 1) * 128], pt)

            def layer_fwd(A_prev, W, bias_col, nk, name):
                """k-outer/m-inner layer: all NM psum banks accumulate at once;
                weight K-tile k is fully consumed after its k-pass."""
                ps = [
                    psp.tile([KM, BS], F32, name=f"ps{name}_{m}", tag="bank")
                    for m in range(NM)
                ]
                for k in range(nk):
                    for m in range(NM):
                        nc.tensor.matmul(
                            ps[m],
                            W[k][:, m * KM : (m + 1) * KM],
                            A_prev[k],
                            start=(k == 0),
                            stop=(k == nk - 1),
                        )
                A_next = []
                for m in range(NM):
                    at = actp.tile(
                        [KM, BS], BF16, name=f"A{name}_{m}", tag=f"a{m}"
                    )
                    nc.scalar.activation(at, ps[m], RELU, bias=bias_col[:, m : m + 1])
                    A_next.append(at)
                return A_next

            # ---- layer 1: 392 -> 1000, ReLU ----
            A_prev = layer_fwd(A0, WIN, bin_t, NH, "1")

            # ---- hidden layers: 1000 -> 1000, ReLU ----
            WH = WH0
            for layer in range(4):
                if layer < 3:
                    WH_next = load_whid(layer + 1)
                else:
                    WH_next = None
                    # last hidden layer in flight: W_out / b_out loads go out now
                    WOUT = []
                    for k in range(NM):
                        wt = woutp.tile(
                            [KM, HALF], BF16, name=f"wo{k}", tag=f"wo{k}"
                        )
                        _gp_dma(nc, wt, w_out[k * KM : (k + 1) * KM, :])
                        WOUT.append(wt)
                    bout_t = biasp.tile([1, HALF], BF16, name="bout", tag="bout")
                    _gp_dma(nc, bout_t, b_out[:].unsqueeze(0))
                A_prev = layer_fwd(A_prev, WH, bhid_t[layer], NM, f"h{layer}")
                WH = WH_next

            # ---- last layer: 1000 -> 392, batch-major output; add to odd cols ----
            ov = out.rearrange("(b p) d -> b p d", p=128)
            for b in range(NB):
                pso = psp.tile([128, HALF], F32, name=f"pso{b}", tag="bank")
                for k in range(NM):
                    nc.tensor.matmul(
                        pso,
                        A_prev[k][:, b * 128 : (b + 1) * 128],
                        WOUT[k],
                        start=(k == 0),
                        stop=False,
                    )
                # bias via ones-row: pso += ones.T @ b_out_row
                nc.tensor.matmul(pso, ones, bout_t, start=False, stop=True)
                xb_odd = X[b].rearrange("p (d two) -> p d two", two=2)[:, :, 1]
                nc.vector.tensor_add(xb_odd, xb_odd, pso)
                eng = nc.sync if b % 2 == 0 else nc.scalar
                eng.dma_start(out=ov[b], in_=X[b])

    nc.finalize()  # Bacc register allocation + freeze (bass2jax won't do it)
    return nc


def get_nc():
    global _CACHED_NC
    if _CACHED_NC is None:
        _CACHED_NC = build_nc()
    return _CACHED_NC


def run(inputs, trace=False, tmpdir=None):
    nc = get_nc()
    f32c = lambda a: np.ascontiguousarray(np.asarray(a), dtype=np.float32)
    x = f32c(inputs["x"])
    shared = {
        k: f32c(inputs[k])
        for k in ("W_in", "b_in", "W_hid", "b_hid", "W_out", "b_out")
    }
    in_maps = [
        {"x": x[i * BS : (i + 1) * BS], **shared} for i in range(N_CORES)
    ]
    res = run_bass_kernel_spmd(
        nc, in_maps, core_ids=list(range(N_CORES)), trace=trace, tmpdir=tmpdir
    )
    y = np.concatenate([res.results[i]["out"] for i in range(N_CORES)], axis=0)
    return y, res


def kernel(**inputs):
    y, _ = run(inputs)
    log_det_J = np.asarray(inputs["log_det_J"], dtype=np.float32)
    return y, log_det_J
